# revision 31
# baseline (speedup 1.0000x reference)
"""Multi-head attention Trainium2 kernel.

Problem: B=4, S=2048, E=512, H=8, D=64 multi-head attention with per-head
Q/K/V projections, softmax (mask is all-ones in this problem), and an
output projection.

Sharding: 8 cores = 4 batches x 2 head-groups (4 heads each). Each core
computes its batch's Q/K/V for its 4 heads, transposed-layout attention,
and a partial output projection (its heads' rows of Wo). The host sums
the two partials per batch and adds the output bias.

Device-side layout notes (per core):
  - x arrives pre-transposed as xT [E, S] (bf16, cast on host).
  - Head pairs are stacked along the partition dim: QT/KT projections run
    with full 128-wide stationary tiles, and the two heads' score matmuls
    occupy disjoint PE row groups (concurrent via tile_position
    auto-derivation) writing the two banks of a shared PSUM tile, so one
    ACTIVATE(Exp) drains both heads per t-tile.
  - Scores are computed transposed (scoresT[t, sq]) so exp output is
    already the K=t moving operand for the attn@V matmul.
  - V carries a ones-column: the softmax denominator appears as row 64 of
    the AV output. The reciprocal row is partition-broadcast (GpSimd) and
    applied during the PSUM->SBUF drain (fused DVE scalar_tensor_tensor),
    so the output projection accumulates all 4 heads in PSUM.
  - The 1/sqrt(D) score scale is folded into Wq/bq on the host.
"""

import os
import numpy as np
import ml_dtypes

B, S, E, H, D = 4, 2048, 512, 8, 64
HPC = 4      # heads per core
NPAIR = 2    # head pairs per core
SQ = 512     # query-chunk width (one fp32 PSUM bank)

_NC_CACHE = {}
LAST_RESULTS = None


def build_nc(s=S):
    """Build the (single-core) Bass program; same program runs SPMD on all 8
    cores with per-core input data."""
    import concourse.bass as bass
    import concourse.mybir as mybir
    from concourse import bacc
    from concourse.tile import TileContext
    from contextlib import ExitStack

    f32 = mybir.dt.float32
    bf16 = mybir.dt.bfloat16
    AF = mybir.ActivationFunctionType
    OP = mybir.AluOpType

    n_sc = s // SQ
    n_tt = s // 128
    n_ec = E // 128

    nc = bacc.Bacc(None, target_bir_lowering=False, debug=False)
    xt_d = nc.dram_tensor("xt", [E, s], bf16, kind="ExternalInput")
    wq_d = nc.dram_tensor("wq", [128, NPAIR * n_ec * 128], bf16, kind="ExternalInput")
    wk_d = nc.dram_tensor("wk", [128, NPAIR * n_ec * 128], bf16, kind="ExternalInput")
    wv_d = nc.dram_tensor("wv", [128, NPAIR * n_ec * 128], bf16, kind="ExternalInput")
    bqk_d = nc.dram_tensor("bqk", [128, 2 * NPAIR], f32, kind="ExternalInput")
    bvb_d = nc.dram_tensor("bvb", [128, HPC * 64], f32, kind="ExternalInput")
    wo2_d = nc.dram_tensor("wo2", [128, NPAIR * 512], bf16, kind="ExternalInput")
    y_d = nc.dram_tensor("y", [s, E], f32, kind="ExternalOutput")

    with TileContext(nc) as tc, ExitStack() as ctx:
        const = ctx.enter_context(tc.tile_pool(name="const", bufs=1))

        # --- static SBUF tensors ---
        xt_sb = const.tile([128, n_ec * s], bf16, name="xt_sb")
        qt_sb = const.tile([128, NPAIR * s], bf16, name="qt_sb")
        kt_sb = const.tile([128, NPAIR * s], bf16, name="kt_sb")
        v_sb = const.tile([128, HPC * n_tt * 65], bf16, name="v_sb")
        wq_sb = const.tile([128, NPAIR * n_ec * 128], bf16, name="wq_sb")
        wk_sb = const.tile([128, NPAIR * n_ec * 128], bf16, name="wk_sb")
        wv_sb = const.tile([128, NPAIR * n_ec * 128], bf16, name="wv_sb")
        bqk_sb = const.tile([128, 2 * NPAIR], f32, name="bqk_sb")
        bvb_sb = const.tile([128, HPC * 64], f32, name="bvb_sb")
        wo2_sb = const.tile([128, NPAIR * 512], bf16, name="wo2_sb")

        # --- input DMAs (weights first; xt split by s-chunk so the first
        # projection matmuls can start as soon as chunk 0 lands) ---
        nc.sync.dma_start(out=wq_sb, in_=wq_d[:, :])
        nc.sync.dma_start(out=wk_sb, in_=wk_d[:, :])
        nc.sync.dma_start(out=wv_sb, in_=wv_d[:, :])
        nc.sync.dma_start(out=bqk_sb, in_=bqk_d[:, :])
        nc.sync.dma_start(out=bvb_sb, in_=bvb_d[:, :])
        nc.sync.dma_start(out=wo2_sb, in_=wo2_d[:, :])
        for c in range(n_sc):
            for ec in range(n_ec):
                nc.sync.dma_start(
                    out=xt_sb[:, ec * s + c * SQ: ec * s + (c + 1) * SQ],
                    in_=xt_d[ec * 128:(ec + 1) * 128, c * SQ:(c + 1) * SQ],
                )

        # ones column of V (denominator trick)
        v_r = v_sb.rearrange("p (n c) -> p n c", c=65)
        nc.vector.memset(v_r[:, :, 64:65], 1.0)

        # "touch" the DMA'd bias tensors from DVE so the DMA wait lands on
        # these (TensorScalar ISA structs carry limited inline sync-waits;
        # real consumers also wait on PE).
        touch = const.tile([128, 2], f32, name="touch")
        nc.vector.tensor_copy(touch[:, 0:1], bqk_sb[:, 0:1])
        nc.vector.tensor_copy(touch[:, 1:2], bvb_sb[:, 0:1])

        # PE-side touches of the weight DMAs (keeps DMA waits off the real
        # matmuls, which also wait on PE/DVE).
        with tc.tile_pool(name="tch", bufs=1, space="PSUM") as tch:
            for i, w in enumerate((wq_sb, wk_sb, wv_sb, wo2_sb)):
                pt = tch.tile([1, 1], f32, name=f"tch{i}", tag="tch")
                nc.tensor.matmul(pt, lhsT=w[:, 0:1], rhs=w[:, 0:1], start=True, stop=True)

        # --- phase 2: a 2-deep software pipeline over slots (c, p).
        # Slot k emits: scores+exp for slot k interleaved per t-tile with
        # the AV matmuls of slot k-1 (whose exps finished a slot ago, so
        # the in-order PE stream never waits on ScalarE), then the pending
        # output projection, then slot k-1's normalize chain.
        # PSUM stack (LIFO): sp(4) | pj(4) -> pv(4) -> av(3)+yp(1).
        sp = ctx.enter_context(tc.tile_pool(name="sp", bufs=2, space="PSUM"))
        ex = ctx.enter_context(tc.tile_pool(name="ex", bufs=3))
        otp = ctx.enter_context(tc.tile_pool(name="ot", bufs=6))
        rrp = ctx.enter_context(tc.tile_pool(name="rr", bufs=4))
        ysp = ctx.enter_context(tc.tile_pool(name="ys", bufs=3))

        slots = [(c, p) for c in range(n_sc) for p in range(NPAIR)]
        AVLAG = 0
        state = {}      # k -> dict(exp2, avps)
        ot2_done = {}   # k -> ot2 tile
        pending_oproj = []
        emitted_oproj = []

        def emit_proj_chunk(p, c, wsb, bcol, dst, pj):
            ps = pj.tile([128, SQ], f32, name="ps", tag="ps")
            for ec in range(n_ec):
                nc.tensor.matmul(
                    ps,
                    lhsT=wsb[:, (p * n_ec + ec) * 128:(p * n_ec + ec + 1) * 128],
                    rhs=xt_sb[:, ec * s + c * SQ: ec * s + (c + 1) * SQ],
                    start=(ec == 0),
                    stop=(ec == n_ec - 1),
                )
            nc.vector.tensor_scalar_add(
                dst[:, p * s + c * SQ: p * s + (c + 1) * SQ],
                ps,
                bqk_sb[:, bcol:bcol + 1],
            )

        def emit_qk(p, pj):
            for c in range(n_sc):
                emit_proj_chunk(p, c, wk_sb, NPAIR + p, kt_sb, pj)
            for c in range(n_sc):
                emit_proj_chunk(p, c, wq_sb, p, qt_sb, pj)

        def emit_scores_tt(k, tt):
            c, p = slots[k]
            spp = sp.tile([128, 1024], f32, name="spp", tag="sp")
            for j in range(2):
                po = j * 64
                nc.tensor.matmul(
                    spp[:, j * 512:(j + 1) * 512],
                    lhsT=kt_sb[po:po + 64, p * s + tt * 128: p * s + (tt + 1) * 128],
                    rhs=qt_sb[po:po + 64, p * s + c * SQ: p * s + (c + 1) * SQ],
                    start=True,
                    stop=True,
                )
            nc.scalar.activation(
                state[k]["exp2"][:, tt * 1024:(tt + 1) * 1024], spp, AF.Exp
            )

        def emit_av_tt(k, tt):
            c, p = slots[k]
            st = state[k]
            if st["avps"] is None:
                st["avps"] = [
                    av.tile([65, 512], f32, name=f"avp{j}", tag="av")
                    for j in range(2)
                ]
            for j in range(2):
                hl = 2 * p + j
                nc.tensor.matmul(
                    st["avps"][j],
                    lhsT=v_sb[:, (hl * n_tt + tt) * 65:(hl * n_tt + tt) * 65 + 65],
                    rhs=st["exp2"][:, tt * 1024 + j * 512: tt * 1024 + (j + 1) * 512],
                    start=(tt == 0),
                    stop=(tt == n_tt - 1),
                )

        def emit_normalize(k):
            # Drain AV PSUM to fp32 SBUF immediately (frees the AV banks so
            # the next slot's AV matmuls never wait on this chain), then:
            # DVE reciprocal of the denominator row, GpSimd partition
            # broadcast, fused multiply+cast back to bf16. All SBUF-side,
            # fully off the PE critical path.
            avps = state[k]["avps"]
            ot2 = otp.tile([128, 512], bf16, name="ot2", tag="ot")
            raws, rrs = [], []
            for j in range(2):
                # both copies emitted first: in the in-order DVE stream the
                # AV PSUM banks free up before any reciprocal work
                raw_t = rrp.tile([65, 512], f32, name="raw", tag="raw")
                nc.vector.tensor_copy(raw_t, avps[j])
                raws.append(raw_t)
            for j in range(2):
                rr_t = rrp.tile([1, 512], f32, name="rr", tag="rr")
                nc.vector.reciprocal(rr_t, raws[j][64:65, :])
                rrs.append(rr_t)
            for j in range(2):
                rb_t = rrp.tile([64, 512], f32, name="rb", tag="rb")
                nc.gpsimd.partition_broadcast(rb_t, rrs[j], channels=64)
                nc.vector.scalar_tensor_tensor(
                    out=ot2[j * 64:(j + 1) * 64, :],
                    in0=raws[j][0:64, :],
                    scalar=1.0,
                    in1=rb_t,
                    op0=OP.mult,
                    op1=OP.mult,
                )
            ot2_done[k] = ot2
            if k % 2 == 1:
                pending_oproj.append((k // 2, k))

        def emit_outproj_group(c_prev, i):
            yp = ypp.tile([128, 512], f32, name="yp", tag="yp")
            for p in range(NPAIR):
                nc.tensor.matmul(
                    yp,
                    lhsT=ot2_done[2 * c_prev + p][:, i * 128:(i + 1) * 128],
                    rhs=wo2_sb[:, p * 512:(p + 1) * 512],
                    start=(p == 0),
                    stop=(p == NPAIR - 1),
                )
            ys_t = ysp.tile([128, 512], f32, name="ys", tag="ys")
            nc.vector.tensor_copy(ys_t, yp)
            nc.sync.dma_start(
                out=y_d[c_prev * SQ + i * 128: c_prev * SQ + (i + 1) * 128, :],
                in_=ys_t,
            )

        def emit_outproj(c_prev):
            for i in range(SQ // 128):
                emit_outproj_group(c_prev, i)
            emitted_oproj.append(c_prev)

        # slot 0 scores run inside the pj scope (sp+pj coexist: 8 banks).
        # K(p0) and Q(p0,c0) come first so slot-0 scores (and ScalarE) start
        # as early as possible; the remaining projections slide under the
        # slot-0 exp cadence.
        state[0] = {"exp2": ex.tile([128, n_tt * 1024], bf16, name="exp2", tag="exp"),
                    "avps": None}
        with tc.tile_pool(name="pj", bufs=4, space="PSUM") as pj:
            for c in range(n_sc):
                emit_proj_chunk(0, c, wk_sb, NPAIR, kt_sb, pj)
            emit_proj_chunk(0, 0, wq_sb, 0, qt_sb, pj)
            for tt in range(n_tt):
                emit_scores_tt(0, tt)
                if tt % 4 == 1 and 1 + tt // 4 < n_sc:
                    emit_proj_chunk(0, 1 + tt // 4, wq_sb, 0, qt_sb, pj)
            for c in range(1 + max(0, (n_tt - 2) // 4 + 1), n_sc):
                emit_proj_chunk(0, c, wq_sb, 0, qt_sb, pj)
            emit_qk(1, pj)

        # V natural [t, d], all 4 heads per matmul (wv packed ec-major so
        # one N=256 moving operand covers both pairs)
        with tc.tile_pool(name="pv", bufs=4, space="PSUM") as pv:
            for tt in range(n_tt):
                ps = pv.tile([128, 256], f32, name="psv", tag="psv")
                for ec in range(n_ec):
                    nc.tensor.matmul(
                        ps,
                        lhsT=xt_sb[:, ec * s + tt * 128: ec * s + (tt + 1) * 128],
                        rhs=wv_sb[:, ec * 256:(ec + 1) * 256],
                        start=(ec == 0),
                        stop=(ec == n_ec - 1),
                    )
                for hl in range(HPC):
                    base = (hl * n_tt + tt) * 65
                    nc.vector.scalar_tensor_tensor(
                        out=v_sb[:, base: base + 64],
                        in0=ps[:, hl * 64:(hl + 1) * 64],
                        scalar=1.0,
                        in1=bvb_sb[:, hl * 64:(hl + 1) * 64],
                        op0=OP.mult,
                        op1=OP.add,
                    )

        av = ctx.enter_context(tc.tile_pool(name="av", bufs=3, space="PSUM"))
        ypp = ctx.enter_context(tc.tile_pool(name="yp", bufs=1, space="PSUM"))

        for k in range(1, len(slots)):
            state[k] = {"exp2": ex.tile([128, n_tt * 1024], bf16, name="exp2", tag="exp"),
                        "avps": None}
            # spread the pending output projection through the late t-tiles
            # (keeps PE idle windows below the ~3.4us HAM re-throttle
            # threshold). Only pick up work normalized >= 2 slots ago so the
            # outproj LDWEIGHTS never races the normalize chain.
            op_c = None
            if pending_oproj and pending_oproj[0][1] + 2 <= k:
                op_c = pending_oproj.pop(0)[0]
            n_grp = SQ // 128
            spread = {n_tt - 2 * n_grp + 1 + 2 * i: i for i in range(n_grp)} \
                if n_tt >= 2 * n_grp else {}
            for tt in range(n_tt):
                emit_scores_tt(k, tt)
                if tt >= AVLAG:
                    emit_av_tt(k - 1, tt - AVLAG)
                if op_c is not None and tt in spread:
                    emit_outproj_group(op_c, spread[tt])
            if op_c is not None:
                for i in range(len(spread), n_grp):
                    emit_outproj_group(op_c, i)
                emitted_oproj.append(op_c)
            for tt in range(n_tt - AVLAG, n_tt):
                emit_av_tt(k - 1, tt)
            emit_normalize(k - 1)
            state.pop(k - 1)

        # pipeline tail: the last slot's AV interleaved with any output
        # projections whose inputs are already normalized
        last = len(slots) - 1
        ready = [c for c, k_at in pending_oproj if k_at < last]
        late = [c for c, k_at in pending_oproj if k_at >= last]
        groups = [(c, i) for c in ready for i in range(SQ // 128)]
        for tt in range(n_tt):
            emit_av_tt(last, tt)
            if tt % 2 == 1 and tt // 2 < len(groups):
                emit_outproj_group(*groups[tt // 2])
        for c, i in groups[max(0, n_tt // 2):]:
            emit_outproj_group(c, i)
        emitted_oproj.extend(ready)
        emit_normalize(last)
        for c, _ in pending_oproj:
            if c not in emitted_oproj:
                emit_outproj(c)
        pending_oproj.clear()
        assert sorted(emitted_oproj) == list(range(n_sc)), emitted_oproj
    nc.compile()
    return nc


def _get_nc(s=S):
    if s not in _NC_CACHE:
        _NC_CACHE[s] = build_nc(s)
    return _NC_CACHE[s]


def make_core_inputs(x_b, Wq4, bq4, Wk4, bk4, Wv4, bv4, Wo4, s=S):
    """Build one core's input map. x_b: [s, E] f32. Wq4/...: this core's 4
    heads ([4, E, D] / [4, D]); Wo4: [4*D, E] rows of Wo for these heads."""
    bf16 = ml_dtypes.bfloat16
    n_ec = E // 128
    scale = 1.0 / np.sqrt(np.float32(D))

    xt = np.ascontiguousarray(x_b.T).astype(bf16)

    def pack_w(W4):
        arr = np.zeros((128, NPAIR * n_ec * 128), np.float32)
        for p in range(NPAIR):
            for ec in range(n_ec):
                blk = arr[:, (p * n_ec + ec) * 128:(p * n_ec + ec + 1) * 128]
                for j in range(2):
                    blk[:, j * 64:(j + 1) * 64] = W4[2 * p + j, ec * 128:(ec + 1) * 128, :]
        return arr

    wq = (pack_w(Wq4) * scale).astype(bf16)
    wk = pack_w(Wk4).astype(bf16)

    # wv: ec-major, all 4 heads per 256-wide block
    wv = np.zeros((128, n_ec * 256), np.float32)
    for ec in range(n_ec):
        for hl in range(HPC):
            wv[:, ec * 256 + hl * 64: ec * 256 + (hl + 1) * 64] = \
                Wv4[hl, ec * 128:(ec + 1) * 128, :]
    wv = wv.astype(bf16)

    bqk = np.zeros((128, 2 * NPAIR), np.float32)
    for p in range(NPAIR):
        bqk[:, p] = np.concatenate([bq4[2 * p], bq4[2 * p + 1]]) * scale
        bqk[:, NPAIR + p] = np.concatenate([bk4[2 * p], bk4[2 * p + 1]])
    bvb = np.tile(np.concatenate([bv4[h] for h in range(HPC)])[None, :], (128, 1)).astype(np.float32)

    wo2 = np.zeros((128, NPAIR * 512), np.float32)
    for p in range(NPAIR):
        wo2[:, p * 512:(p + 1) * 512] = Wo4[p * 128:(p + 1) * 128, :]
    wo2 = wo2.astype(bf16)

    return {
        "xt": xt, "wq": wq, "wk": wk, "wv": wv,
        "bqk": bqk, "bvb": bvb, "wo2": wo2,
    }


def kernel(**inputs):
    global LAST_RESULTS
    from concourse.bass_utils import run_bass_kernel_spmd

    x = np.asarray(inputs["x"], np.float32)
    Wq = np.asarray(inputs["Wq"], np.float32)
    bq = np.asarray(inputs["bq"], np.float32)
    Wk = np.asarray(inputs["Wk"], np.float32)
    bk = np.asarray(inputs["bk"], np.float32)
    Wv = np.asarray(inputs["Wv"], np.float32)
    bv = np.asarray(inputs["bv"], np.float32)
    Wo = np.asarray(inputs["Wo"], np.float32)
    bo = np.asarray(inputs["bo"], np.float32)

    nc = _get_nc()
    in_maps = []
    for c in range(2 * B):
        b, g = c // 2, c % 2
        hs = slice(4 * g, 4 * g + 4)
        in_maps.append(make_core_inputs(
            x[b], Wq[hs], bq[hs], Wk[hs], bk[hs], Wv[hs], bv[hs],
            Wo[4 * g * 64:(4 * g + 4) * 64, :],
        ))

    trace = bool(int(os.environ.get("BASS_KERNEL_TRACE", "0")))
    res = run_bass_kernel_spmd(nc, in_maps, core_ids=list(range(2 * B)), trace=trace)
    LAST_RESULTS = res

    y = np.zeros((B, S, E), np.float32)
    for b in range(B):
        y[b] = res.results[2 * b]["y"] + res.results[2 * b + 1]["y"] + bo[None, :]
    return y


# revision 35
# speedup vs baseline: 1.0178x; 1.0178x over previous
"""Multi-head attention Trainium2 kernel.

Problem: B=4, S=2048, E=512, H=8, D=64 multi-head attention with per-head
Q/K/V projections, softmax (mask is all-ones in this problem), and an
output projection.

Sharding: 8 cores = 4 batches x 2 head-groups (4 heads each). Each core
computes its batch's Q/K/V for its 4 heads, transposed-layout attention,
and a partial output projection (its heads' rows of Wo). The host sums
the two partials per batch and adds the output bias.

Device-side layout notes (per core):
  - x arrives pre-transposed as xT [E, S] (bf16, cast on host).
  - Head pairs are stacked along the partition dim: QT/KT projections run
    with full 128-wide stationary tiles, and the two heads' score matmuls
    occupy disjoint PE row groups (concurrent via tile_position
    auto-derivation) writing the two banks of a shared PSUM tile, so one
    ACTIVATE(Exp) drains both heads per t-tile.
  - Scores are computed transposed (scoresT[t, sq]) so exp output is
    already the K=t moving operand for the attn@V matmul.
  - V carries a ones-column: the softmax denominator appears as row 64 of
    the AV output. The reciprocal row is partition-broadcast (GpSimd) and
    applied during the PSUM->SBUF drain (fused DVE scalar_tensor_tensor),
    so the output projection accumulates all 4 heads in PSUM.
  - The 1/sqrt(D) score scale is folded into Wq/bq on the host.
"""

import os
import numpy as np
import ml_dtypes

B, S, E, H, D = 4, 2048, 512, 8, 64
HPC = 4      # heads per core
NPAIR = 2    # head pairs per core
SQ = 512     # query-chunk width (one fp32 PSUM bank)

_NC_CACHE = {}
LAST_RESULTS = None


def build_nc(s=S):
    """Build the (single-core) Bass program; same program runs SPMD on all 8
    cores with per-core input data."""
    import concourse.bass as bass
    import concourse.mybir as mybir
    from concourse import bacc
    from concourse.tile import TileContext
    from contextlib import ExitStack

    f32 = mybir.dt.float32
    bf16 = mybir.dt.bfloat16
    AF = mybir.ActivationFunctionType
    OP = mybir.AluOpType

    n_sc = s // SQ
    n_tt = s // 128
    n_ec = E // 128

    nc = bacc.Bacc(None, target_bir_lowering=False, debug=False)
    xt_d = nc.dram_tensor("xt", [E, s], bf16, kind="ExternalInput")
    wq_d = nc.dram_tensor("wq", [128, NPAIR * n_ec * 128], bf16, kind="ExternalInput")
    wk_d = nc.dram_tensor("wk", [128, NPAIR * n_ec * 128], bf16, kind="ExternalInput")
    wv_d = nc.dram_tensor("wv", [128, NPAIR * n_ec * 128], bf16, kind="ExternalInput")
    bqk_d = nc.dram_tensor("bqk", [128, 2 * NPAIR], f32, kind="ExternalInput")
    bvb_d = nc.dram_tensor("bvb", [128, HPC * 64], f32, kind="ExternalInput")
    wo2_d = nc.dram_tensor("wo2", [128, NPAIR * 512], bf16, kind="ExternalInput")
    y_d = nc.dram_tensor("y", [s, E], f32, kind="ExternalOutput")

    with TileContext(nc) as tc, ExitStack() as ctx:
        const = ctx.enter_context(tc.tile_pool(name="const", bufs=1))

        # --- static SBUF tensors ---
        xt_sb = const.tile([128, n_ec * s], bf16, name="xt_sb")
        qt_sb = const.tile([128, NPAIR * s], bf16, name="qt_sb")
        kt_sb = const.tile([128, NPAIR * s], bf16, name="kt_sb")
        v_sb = const.tile([128, HPC * n_tt * 65], bf16, name="v_sb")
        wq_sb = const.tile([128, NPAIR * n_ec * 128], bf16, name="wq_sb")
        wk_sb = const.tile([128, NPAIR * n_ec * 128], bf16, name="wk_sb")
        wv_sb = const.tile([128, NPAIR * n_ec * 128], bf16, name="wv_sb")
        bqk_sb = const.tile([128, 2 * NPAIR], f32, name="bqk_sb")
        bvb_sb = const.tile([128, HPC * 64], f32, name="bvb_sb")
        wo2_sb = const.tile([128, NPAIR * 512], bf16, name="wo2_sb")

        # --- input DMAs (weights first; xt split by s-chunk so the first
        # projection matmuls can start as soon as chunk 0 lands) ---
        nc.sync.dma_start(out=wq_sb, in_=wq_d[:, :])
        nc.sync.dma_start(out=wk_sb, in_=wk_d[:, :])
        nc.sync.dma_start(out=wv_sb, in_=wv_d[:, :])
        nc.sync.dma_start(out=bqk_sb, in_=bqk_d[:, :])
        nc.sync.dma_start(out=bvb_sb, in_=bvb_d[:, :])
        nc.sync.dma_start(out=wo2_sb, in_=wo2_d[:, :])
        for c in range(n_sc):
            for ec in range(n_ec):
                nc.sync.dma_start(
                    out=xt_sb[:, ec * s + c * SQ: ec * s + (c + 1) * SQ],
                    in_=xt_d[ec * 128:(ec + 1) * 128, c * SQ:(c + 1) * SQ],
                )

        # ones column of V (denominator trick)
        v_r = v_sb.rearrange("p (n c) -> p n c", c=65)
        nc.vector.memset(v_r[:, :, 64:65], 1.0)

        # "touch" the DMA'd bias tensors from DVE so the DMA wait lands on
        # these (TensorScalar ISA structs carry limited inline sync-waits;
        # real consumers also wait on PE).
        touch = const.tile([128, 2], f32, name="touch")
        nc.vector.tensor_copy(touch[:, 0:1], bqk_sb[:, 0:1])
        nc.vector.tensor_copy(touch[:, 1:2], bvb_sb[:, 0:1])

        # PE-side touches of the weight DMAs (keeps DMA waits off the real
        # matmuls, which also wait on PE/DVE).
        with tc.tile_pool(name="tch", bufs=1, space="PSUM") as tch:
            for i, w in enumerate((wq_sb, wk_sb, wv_sb, wo2_sb)):
                pt = tch.tile([1, 1], f32, name=f"tch{i}", tag="tch")
                nc.tensor.matmul(pt, lhsT=w[:, 0:1], rhs=w[:, 0:1], start=True, stop=True)

        # --- phase 2: a 2-deep software pipeline over slots (c, p).
        # Slot k emits: scores+exp for slot k interleaved per t-tile with
        # the AV matmuls of slot k-1 (whose exps finished a slot ago, so
        # the in-order PE stream never waits on ScalarE), then the pending
        # output projection, then slot k-1's normalize chain.
        # PSUM stack (LIFO): sp(4) | pj(4) -> pv(4) -> av(3)+yp(1).
        sp = ctx.enter_context(tc.tile_pool(name="sp", bufs=2, space="PSUM"))
        ex = ctx.enter_context(tc.tile_pool(name="ex", bufs=3))
        otp = ctx.enter_context(tc.tile_pool(name="ot", bufs=6))
        rrp = ctx.enter_context(tc.tile_pool(name="rr", bufs=4))
        ysp = ctx.enter_context(tc.tile_pool(name="ys", bufs=3))

        slots = [(c, p) for c in range(n_sc) for p in range(NPAIR)]
        AVLAG = 0
        state = {}      # k -> dict(exp2, avps)
        ot2_done = {}   # k -> ot2 tile
        pending_oproj = []
        emitted_oproj = []

        def emit_proj_chunk(p, c, wsb, bcol, dst, pj):
            ps = pj.tile([128, SQ], f32, name="ps", tag="ps")
            for ec in range(n_ec):
                nc.tensor.matmul(
                    ps,
                    lhsT=wsb[:, (p * n_ec + ec) * 128:(p * n_ec + ec + 1) * 128],
                    rhs=xt_sb[:, ec * s + c * SQ: ec * s + (c + 1) * SQ],
                    start=(ec == 0),
                    stop=(ec == n_ec - 1),
                )
            nc.vector.tensor_scalar_add(
                dst[:, p * s + c * SQ: p * s + (c + 1) * SQ],
                ps,
                bqk_sb[:, bcol:bcol + 1],
            )

        def emit_qk(p, pj):
            for c in range(n_sc):
                emit_proj_chunk(p, c, wk_sb, NPAIR + p, kt_sb, pj)
            for c in range(n_sc):
                emit_proj_chunk(p, c, wq_sb, p, qt_sb, pj)

        def emit_scores_tt(k, tt):
            c, p = slots[k]
            spp = sp.tile([128, 1024], f32, name="spp", tag="sp")
            for j in range(2):
                po = j * 64
                nc.tensor.matmul(
                    spp[:, j * 512:(j + 1) * 512],
                    lhsT=kt_sb[po:po + 64, p * s + tt * 128: p * s + (tt + 1) * 128],
                    rhs=qt_sb[po:po + 64, p * s + c * SQ: p * s + (c + 1) * SQ],
                    start=True,
                    stop=True,
                )
            nc.scalar.activation(
                state[k]["exp2"][:, tt * 1024:(tt + 1) * 1024], spp, AF.Exp
            )

        def emit_av_tt(k, tt):
            c, p = slots[k]
            st = state[k]
            if st["avps"] is None:
                st["avps"] = [
                    av.tile([65, 512], f32, name=f"avp{j}", tag="av")
                    for j in range(2)
                ]
            for j in range(2):
                hl = 2 * p + j
                nc.tensor.matmul(
                    st["avps"][j],
                    lhsT=v_sb[:, (hl * n_tt + tt) * 65:(hl * n_tt + tt) * 65 + 65],
                    rhs=st["exp2"][:, tt * 1024 + j * 512: tt * 1024 + (j + 1) * 512],
                    start=(tt == 0),
                    stop=(tt == n_tt - 1),
                )

        def emit_normalize(k, use_act=False):
            # Drain AV PSUM to fp32 SBUF immediately (frees the AV banks so
            # the next slot's AV matmuls never wait on this chain), then:
            # DVE reciprocal of the denominator row, GpSimd partition
            # broadcast, fused multiply+cast back to bf16. All SBUF-side,
            # fully off the PE critical path.
            avps = state[k]["avps"]
            ot2 = otp.tile([128, 512], bf16, name="ot2", tag="ot")
            raws, rrs = [], []
            for j in range(2):
                # both copies emitted first: in the in-order DVE stream the
                # AV PSUM banks free up before any reciprocal work
                raw_t = rrp.tile([65, 512], f32, name="raw", tag="raw")
                nc.vector.tensor_copy(raw_t, avps[j])
                raws.append(raw_t)
            for j in range(2):
                rr_t = rrp.tile([1, 512], f32, name="rr", tag="rr")
                if use_act:
                    # tail only: ScalarE is idle there and exp(-ln(x)) is
                    # ~2.8x faster than the one-lane DVE reciprocal
                    ln_t = rrp.tile([1, 512], f32, name="ln", tag="ln")
                    nc.scalar.activation(ln_t, raws[j][64:65, :], AF.Ln)
                    nc.scalar.activation(rr_t, ln_t, AF.Exp, scale=-1.0)
                else:
                    nc.vector.reciprocal(rr_t, raws[j][64:65, :])
                rrs.append(rr_t)
            for j in range(2):
                rb_t = rrp.tile([64, 512], f32, name="rb", tag="rb")
                nc.gpsimd.partition_broadcast(rb_t, rrs[j], channels=64)
                nc.vector.scalar_tensor_tensor(
                    out=ot2[j * 64:(j + 1) * 64, :],
                    in0=raws[j][0:64, :],
                    scalar=1.0,
                    in1=rb_t,
                    op0=OP.mult,
                    op1=OP.mult,
                )
            ot2_done[k] = ot2
            if k % 2 == 1:
                pending_oproj.append((k // 2, k))

        def emit_outproj_group(c_prev, i):
            yp = ypp.tile([128, 512], f32, name="yp", tag="yp")
            for p in range(NPAIR):
                nc.tensor.matmul(
                    yp,
                    lhsT=ot2_done[2 * c_prev + p][:, i * 128:(i + 1) * 128],
                    rhs=wo2_sb[:, p * 512:(p + 1) * 512],
                    start=(p == 0),
                    stop=(p == NPAIR - 1),
                )
            ys_t = ysp.tile([128, 512], f32, name="ys", tag="ys")
            nc.vector.tensor_copy(ys_t, yp)
            nc.sync.dma_start(
                out=y_d[c_prev * SQ + i * 128: c_prev * SQ + (i + 1) * 128, :],
                in_=ys_t,
            )

        def emit_outproj(c_prev):
            for i in range(SQ // 128):
                emit_outproj_group(c_prev, i)
            emitted_oproj.append(c_prev)

        # slot 0 scores run inside the pj scope (sp+pj coexist: 8 banks).
        # K(p0) and Q(p0,c0) come first so slot-0 scores (and ScalarE) start
        # as early as possible; the remaining projections slide under the
        # slot-0 exp cadence.
        state[0] = {"exp2": ex.tile([128, n_tt * 1024], bf16, name="exp2", tag="exp"),
                    "avps": None}
        with tc.tile_pool(name="pj", bufs=4, space="PSUM") as pj:
            for c in range(n_sc):
                emit_proj_chunk(0, c, wk_sb, NPAIR, kt_sb, pj)
            emit_proj_chunk(0, 0, wq_sb, 0, qt_sb, pj)
            for tt in range(n_tt):
                emit_scores_tt(0, tt)
                if tt % 4 == 1 and 1 + tt // 4 < n_sc:
                    emit_proj_chunk(0, 1 + tt // 4, wq_sb, 0, qt_sb, pj)
            for c in range(1 + max(0, (n_tt - 2) // 4 + 1), n_sc):
                emit_proj_chunk(0, c, wq_sb, 0, qt_sb, pj)
            emit_qk(1, pj)

        # V natural [t, d], all 4 heads per matmul (wv packed ec-major so
        # one N=256 moving operand covers both pairs)
        with tc.tile_pool(name="pv", bufs=4, space="PSUM") as pv:
            for tt in range(n_tt):
                ps = pv.tile([128, 256], f32, name="psv", tag="psv")
                for ec in range(n_ec):
                    nc.tensor.matmul(
                        ps,
                        lhsT=xt_sb[:, ec * s + tt * 128: ec * s + (tt + 1) * 128],
                        rhs=wv_sb[:, ec * 256:(ec + 1) * 256],
                        start=(ec == 0),
                        stop=(ec == n_ec - 1),
                    )
                for hl in range(HPC):
                    base = (hl * n_tt + tt) * 65
                    nc.vector.scalar_tensor_tensor(
                        out=v_sb[:, base: base + 64],
                        in0=ps[:, hl * 64:(hl + 1) * 64],
                        scalar=1.0,
                        in1=bvb_sb[:, hl * 64:(hl + 1) * 64],
                        op0=OP.mult,
                        op1=OP.add,
                    )

        av = ctx.enter_context(tc.tile_pool(name="av", bufs=3, space="PSUM"))
        ypp = ctx.enter_context(tc.tile_pool(name="yp", bufs=1, space="PSUM"))

        for k in range(1, len(slots)):
            state[k] = {"exp2": ex.tile([128, n_tt * 1024], bf16, name="exp2", tag="exp"),
                        "avps": None}
            # spread the pending output projection through the late t-tiles
            # (keeps PE idle windows below the ~3.4us HAM re-throttle
            # threshold). Only pick up work normalized >= 2 slots ago so the
            # outproj LDWEIGHTS never races the normalize chain.
            op_c = None
            if pending_oproj and pending_oproj[0][1] + 3 <= k:
                op_c = pending_oproj.pop(0)[0]
            n_grp = SQ // 128
            spread = {n_tt - 2 * n_grp + 1 + 2 * i: i for i in range(n_grp)} \
                if n_tt >= 2 * n_grp else {}
            for tt in range(n_tt):
                emit_scores_tt(k, tt)
                if tt >= AVLAG:
                    emit_av_tt(k - 1, tt - AVLAG)
                if op_c is not None and tt in spread:
                    emit_outproj_group(op_c, spread[tt])
            if op_c is not None:
                for i in range(len(spread), n_grp):
                    emit_outproj_group(op_c, i)
                emitted_oproj.append(op_c)
            for tt in range(n_tt - AVLAG, n_tt):
                emit_av_tt(k - 1, tt)
            emit_normalize(k - 1)
            state.pop(k - 1)

        # pipeline tail: the last slot's AV interleaved with any output
        # projections whose inputs are already normalized
        last = len(slots) - 1
        ready = [c for c, k_at in pending_oproj if k_at < last]
        late = [c for c, k_at in pending_oproj if k_at >= last]
        groups = [(c, i) for c in ready for i in range(SQ // 128)]
        for tt in range(n_tt):
            emit_av_tt(last, tt)
            if tt % 2 == 1 and tt // 2 < len(groups):
                emit_outproj_group(*groups[tt // 2])
        for c, i in groups[max(0, n_tt // 2):]:
            emit_outproj_group(c, i)
        emitted_oproj.extend(ready)
        emit_normalize(last, use_act=True)
        for c, _ in pending_oproj:
            if c not in emitted_oproj:
                emit_outproj(c)
        pending_oproj.clear()
        assert sorted(emitted_oproj) == list(range(n_sc)), emitted_oproj
    nc.compile()
    return nc


def _get_nc(s=S):
    if s not in _NC_CACHE:
        _NC_CACHE[s] = build_nc(s)
    return _NC_CACHE[s]


def make_core_inputs(x_b, Wq4, bq4, Wk4, bk4, Wv4, bv4, Wo4, s=S):
    """Build one core's input map. x_b: [s, E] f32. Wq4/...: this core's 4
    heads ([4, E, D] / [4, D]); Wo4: [4*D, E] rows of Wo for these heads."""
    bf16 = ml_dtypes.bfloat16
    n_ec = E // 128
    scale = 1.0 / np.sqrt(np.float32(D))

    xt = np.ascontiguousarray(x_b.T).astype(bf16)

    def pack_w(W4):
        arr = np.zeros((128, NPAIR * n_ec * 128), np.float32)
        for p in range(NPAIR):
            for ec in range(n_ec):
                blk = arr[:, (p * n_ec + ec) * 128:(p * n_ec + ec + 1) * 128]
                for j in range(2):
                    blk[:, j * 64:(j + 1) * 64] = W4[2 * p + j, ec * 128:(ec + 1) * 128, :]
        return arr

    wq = (pack_w(Wq4) * scale).astype(bf16)
    wk = pack_w(Wk4).astype(bf16)

    # wv: ec-major, all 4 heads per 256-wide block
    wv = np.zeros((128, n_ec * 256), np.float32)
    for ec in range(n_ec):
        for hl in range(HPC):
            wv[:, ec * 256 + hl * 64: ec * 256 + (hl + 1) * 64] = \
                Wv4[hl, ec * 128:(ec + 1) * 128, :]
    wv = wv.astype(bf16)

    bqk = np.zeros((128, 2 * NPAIR), np.float32)
    for p in range(NPAIR):
        bqk[:, p] = np.concatenate([bq4[2 * p], bq4[2 * p + 1]]) * scale
        bqk[:, NPAIR + p] = np.concatenate([bk4[2 * p], bk4[2 * p + 1]])
    bvb = np.tile(np.concatenate([bv4[h] for h in range(HPC)])[None, :], (128, 1)).astype(np.float32)

    wo2 = np.zeros((128, NPAIR * 512), np.float32)
    for p in range(NPAIR):
        wo2[:, p * 512:(p + 1) * 512] = Wo4[p * 128:(p + 1) * 128, :]
    wo2 = wo2.astype(bf16)

    return {
        "xt": xt, "wq": wq, "wk": wk, "wv": wv,
        "bqk": bqk, "bvb": bvb, "wo2": wo2,
    }


def kernel(**inputs):
    global LAST_RESULTS
    from concourse.bass_utils import run_bass_kernel_spmd

    x = np.asarray(inputs["x"], np.float32)
    Wq = np.asarray(inputs["Wq"], np.float32)
    bq = np.asarray(inputs["bq"], np.float32)
    Wk = np.asarray(inputs["Wk"], np.float32)
    bk = np.asarray(inputs["bk"], np.float32)
    Wv = np.asarray(inputs["Wv"], np.float32)
    bv = np.asarray(inputs["bv"], np.float32)
    Wo = np.asarray(inputs["Wo"], np.float32)
    bo = np.asarray(inputs["bo"], np.float32)

    nc = _get_nc()
    in_maps = []
    for c in range(2 * B):
        b, g = c // 2, c % 2
        hs = slice(4 * g, 4 * g + 4)
        in_maps.append(make_core_inputs(
            x[b], Wq[hs], bq[hs], Wk[hs], bk[hs], Wv[hs], bv[hs],
            Wo[4 * g * 64:(4 * g + 4) * 64, :],
        ))

    trace = bool(int(os.environ.get("BASS_KERNEL_TRACE", "0")))
    res = run_bass_kernel_spmd(nc, in_maps, core_ids=list(range(2 * B)), trace=trace)
    LAST_RESULTS = res

    y = np.zeros((B, S, E), np.float32)
    for b in range(B):
        y[b] = res.results[2 * b]["y"] + res.results[2 * b + 1]["y"] + bo[None, :]
    return y
